# revision 12
# baseline (speedup 1.0000x reference)
"""Trainium2 Bass kernel for a DeepSpeech2-style CNN-BiLSTM (nn_CNNLSTM).

8 NeuronCores, one uniform SPMD program; every per-core difference is data:
  - Conv frontend: data-parallel, 1 sample/core, freq-kernel x 4-fo-group
    packed matmuls; batch-norm batch stats combined with a tiny AllGather.
  - 5 BiLSTM layers: cores 0-3 forward scans, cores 4-7 backward scans of the
    same 2 samples. wh stays SBUF-resident; each recurrent step is 144
    weight-stationary bf16 matmuls (transposed gate layout, moving N=2) with
    per-chunk gate nonlinearities hidden behind the PE stream.
  - flipseq (per-sample valid-length flip) is an exact 0/1 permutation matmul.
  - Layer boundaries: bf16 h buffers AllGather'd; inputs re-assembled with
    strided DMA access patterns + permutation matmuls, running-BN folded in.
  - Head (BN + FC + log_softmax) on every core for its 2 samples; host takes
    cores 0-3.

kernel(**inputs) -> (log_probs (8,512,29) float32, out_len (8,) int32)
"""

import numpy as np
import ml_dtypes

import concourse.bacc as bacc
import concourse.tile as tile
from concourse import mybir
from concourse.bass_types import AP
from concourse.bass_utils import run_bass_kernel_spmd

BF16 = ml_dtypes.bfloat16
F32 = mybir.dt.float32
BF = mybir.dt.bfloat16
AF = mybir.ActivationFunctionType
AL = mybir.AluOpType
AX = mybir.AxisListType
ET = mybir.EngineType

B, D, T = 8, 161, 1024
TC = 512
F0, F0P = 1312, 1408
HID, G = 768, 3072
NCLS = 29
NC_ = 8
EPS = 1e-5
KC = 6
NCH = 24
FG1, FG2 = 21, 11
K1C = 5
CNT1 = float(B * 81 * TC)
CNT2 = float(B * 41 * TC)
U = 4
HBLK = 128 * (KC * TC * 2)
XBLK = F0P * TC

_CACHE = {}


# ---------------------------------------------------------------- host packing

def _bf(x):
    return np.ascontiguousarray(np.asarray(x, np.float32).astype(BF16))


def _f32(x):
    return np.ascontiguousarray(np.asarray(x, np.float32))


def _pack_conv1(w1, b1):
    w1 = np.asarray(w1, np.float32)
    W = np.zeros((11, 64, 128), np.float32)
    for kt in range(11):
        for d in range(4):
            for kf in range(41):
                W[kt, 2 * d + kf, d * 32:(d + 1) * 32] = w1[kf, kt, 0, :]
    bias = np.tile(np.asarray(b1, np.float32), 4).reshape(128, 1)
    return _bf(W.reshape(11 * 64, 128)), _f32(bias)


def _pack_conv2(w2, b2):
    w2 = np.asarray(w2, np.float32)
    W = np.zeros((11, 7, 128, 128), np.float32)
    for kt in range(11):
        for c7 in range(7):
            for q in range(4):
                for d in range(4):
                    kf = 4 * c7 + q - 2 * d
                    if 0 <= kf <= 20:
                        W[kt, c7, q * 32:(q + 1) * 32, d * 32:(d + 1) * 32] = \
                            w2[kf, kt, :, :]
    bias = np.tile(np.asarray(b2, np.float32), 4).reshape(128, 1)
    return _bf(W.reshape(11 * 7 * 128, 128)), _f32(bias)


def _gate_reorder(w):
    i, f, g, o = np.split(np.asarray(w, np.float32), 4, axis=-1)
    return np.concatenate([i, f, o, g], -1)


def _perm_mat(L):
    pm = np.zeros((TC, TC), np.float32)
    for tau in range(TC):
        t = L - 1 - tau if tau < L else TC - 1 + L - tau
        pm[t, tau] = 1.0
    return pm


def _mask_conv(lv, nfg, nfo):
    m = np.zeros((128, nfg), np.float32)
    for d in range(4):
        for fg in range(nfg):
            fo = 4 * fg + d
            m[d * 32:(d + 1) * 32, fg] = 1.0 if (fo < nfo and fo >= lv) else 0.0
    return np.repeat(m, TC, axis=1)


def _host_pack(inputs):
    lengths = np.asarray(inputs["lengths"]).astype(np.int64)
    out_len = (lengths - 1) // 2 + 1

    w1e, b1e = _pack_conv1(inputs["conv1_w"], inputs["conv1_b"])
    w2e, b2e = _pack_conv2(inputs["conv2_w"], inputs["conv2_b"])

    wi_d, wh_d, bl_d, wi0_d = [[], []], [[], []], [[], []], []
    for d in range(2):
        w0 = np.zeros((F0P, G), np.float32)
        w0[:F0] = _gate_reorder(inputs["lstm0_wi"][d])
        wi0_d.append(w0)
        wh_d[d].append(_gate_reorder(inputs["lstm0_wh"][d]))
        bl_d[d].append(_gate_reorder(inputs["lstm0_b"][d]))
        for i in range(4):
            wi_d[d].append(_gate_reorder(inputs["lstm_wi"][i][d]))
            wh_d[d].append(_gate_reorder(inputs["lstm_wh"][i][d]))
            bl_d[d].append(_gate_reorder(inputs["lstm_b"][i][d]))

    As, Bs = [], []
    for i in range(4):
        A = np.asarray(inputs["rnn_bn_scale"][i], np.float32) / np.sqrt(
            np.asarray(inputs["rnn_bn_var"][i], np.float32) + EPS)
        Bv = np.asarray(inputs["rnn_bn_bias"][i], np.float32) - \
            np.asarray(inputs["rnn_bn_mean"][i], np.float32) * A
        As.append(A), Bs.append(Bv)
    A = np.asarray(inputs["fc_bn_scale"], np.float32) / np.sqrt(
        np.asarray(inputs["fc_bn_var"], np.float32) + EPS)
    Bv = np.asarray(inputs["fc_bn_bias"], np.float32) - \
        np.asarray(inputs["fc_bn_mean"], np.float32) * A
    As.append(A), Bs.append(Bv)
    bnA, bnB = _f32(np.stack(As)), _f32(np.stack(Bs))

    fcw = np.zeros((HID, 32), np.float32)
    fcw[:, :NCLS] = np.asarray(inputs["fc_w"], np.float32)
    sel4 = np.zeros((32, 128), np.float32)
    for c in range(32):
        sel4[c, [c, 32 + c, 64 + c, 96 + c]] = 1.0
    sel4T = np.ascontiguousarray(sel4.T)
    eye32 = np.eye(32, dtype=np.float32)
    eyeb = np.eye(128, dtype=np.float32)

    eye_t = np.eye(TC, dtype=np.float32)
    in_maps = []
    for c in range(NC_):
        is_fwd = c < 4
        d = 0 if is_fwd else 1
        samples = [2 * (c % 4), 2 * (c % 4) + 1]
        pm = np.zeros((6, 2, 2, TC, TC), np.float32)
        for si, gs in enumerate(samples):
            pmat = _perm_mat(int(out_len[gs]))
            pm[0, 0, si] = eye_t if is_fwd else pmat
            for r in range(1, 5):
                pm[r, 0, si] = eye_t if is_fwd else pmat
                pm[r, 1, si] = pmat if is_fwd else eye_t
            pm[5, 0, si] = eye_t
            pm[5, 1, si] = pmat
        m = {
            "xs": _f32(np.asarray(inputs["inputs"])[c, 0]),
            "w1e": w1e, "b1e": b1e, "w2e": w2e, "b2e": b2e,
            "mask1": _bf(_mask_conv(int(lengths[c]), FG1, 81)),
            "mask2": _bf(_mask_conv(int(lengths[c]), FG2, 41)),
            "sel4": sel4, "sel4T": sel4T, "eye32": eye32,
            "eyeb": _bf(eyeb),
            "wi0": _bf(wi0_d[d]),
            "wi": _bf(np.concatenate(wi_d[d], 0)),
            "wh": _bf(np.concatenate(wh_d[d], 0)),
            "bl": _f32(np.stack(bl_d[d])),
            "bnA": bnA, "bnB": bnB,
            "pmats": _bf(pm.reshape(6 * 2 * 2 * TC, TC)),
            "fcw": _bf(fcw),
        }
        in_maps.append(m)
    return in_maps, out_len.astype(np.asarray(inputs["lengths"]).dtype)


# ------------------------------------------------------------- device program

PHASES = 99


def _build_program(debug=False):
    nc = bacc.Bacc("TRN2", target_bir_lowering=False, debug=False,
                   num_devices=NC_, enable_asserts=False)
    E = {}
    for name, shape, dt in [
        ("xs", [D, T], F32), ("w1e", [11 * 64, 128], BF), ("b1e", [128, 1], F32),
        ("w2e", [11 * 7 * 128, 128], BF), ("b2e", [128, 1], F32),
        ("mask1", [128, FG1 * TC], BF), ("mask2", [128, FG2 * TC], BF),
        ("sel4", [32, 128], F32), ("sel4T", [128, 32], F32),
        ("eye32", [32, 32], F32), ("eyeb", [128, 128], BF),
        ("wi0", [F0P, G], BF), ("wi", [4 * HID, G], BF),
        ("wh", [5 * HID, G], BF), ("bl", [5, G], F32),
        ("bnA", [5, HID], F32), ("bnB", [5, HID], F32),
        ("pmats", [6 * 2 * 2 * TC, TC], BF), ("fcw", [HID, 32], BF),
    ]:
        E[name] = nc.dram_tensor(name, shape, dt, kind="ExternalInput")

    out_lp = nc.dram_tensor("out_lp", [2 * TC, NCLS], F32, kind="ExternalOutput")
    dbg = {}
    if debug:
        dbg["x1f"] = nc.dram_tensor("dbg_x1f", [128, FG1 * TC], BF, kind="ExternalOutput")
        dbg["x2f"] = nc.dram_tensor("dbg_x2f", [128, FG2 * TC], BF, kind="ExternalOutput")
        dbg["x0"] = nc.dram_tensor("dbg_x0", [F0P, TC], BF, kind="ExternalOutput")
        dbg["xw"] = nc.dram_tensor("dbg_xw", [5 * 128, NCH * TC * 2], BF, kind="ExternalOutput")
        dbg["hb"] = nc.dram_tensor("dbg_hb", [5 * 128, KC * TC * 2], BF, kind="ExternalOutput")

    x_pad = nc.dram_tensor("x_pad", [208, 1040], BF, kind="Internal")
    x1p = nc.dram_tensor("x1p", [112 * 32, 522], BF, kind="Internal")
    x0_self = nc.dram_tensor("x0_self", [F0P, TC], BF, kind="Internal")
    x0_ag = nc.dram_tensor("x0_ag", [NC_ * F0P, TC], BF, kind="Internal",
                           addr_space="Shared")
    st_in = [nc.dram_tensor(f"st_in{i}", [128, 2], F32, kind="Internal")
             for i in range(2)]
    st_ag = [nc.dram_tensor(f"st_ag{i}", [NC_ * 128, 2], F32, kind="Internal",
                            addr_space="Shared") for i in range(2)]
    h_in = nc.dram_tensor("h_in", [128, KC * TC * 2], BF, kind="Internal")
    h_ag = [nc.dram_tensor(f"h_ag{l}", [NC_ * 128, KC * TC * 2], BF,
                           kind="Internal", addr_space="Shared")
            for l in range(5)]
    RG = [list(range(NC_))]

    with tile.TileContext(nc) as tc:
        with (
            tc.tile_pool(name="big", bufs=1) as big,
            tc.tile_pool(name="work", bufs=2) as work,
            tc.tile_pool(name="ws", bufs=2) as ws,
            tc.tile_pool(name="ext", bufs=8) as extp,
            tc.tile_pool(name="xsc", bufs=13) as xscp,
            tc.tile_pool(name="sm", bufs=4) as sm,
            tc.tile_pool(name="st", bufs=1) as stp,
            tc.tile_pool(name="psA", bufs=2, space="PSUM") as psA,
            tc.tile_pool(name="psB", bufs=2, space="PSUM") as psB,
            tc.tile_pool(name="psC", bufs=2, space="PSUM") as psC,
        ):
            pid = nc.partition_id()
            _conv(nc, E, big, work, sm, psA, psC,
                  x_pad, x1p, x0_self, st_in, st_ag, RG, dbg)
            nc.gpsimd.collective_compute(
                "AllGather", AL.bypass, replica_groups=RG,
                ins=[x0_self[:, :]], outs=[x0_ag[:, :]])
            if PHASES >= 1:
                _lstm(nc, tc, E, pid, big, work, ws, extp, xscp, sm, stp,
                      psA, psB, psC, x0_ag, h_in, h_ag, RG, out_lp, dbg)
    nc.compile()
    return nc


# ---------------------------------------------------------------------- conv

def _conv(nc, E, big, work, sm, psA, psC,
          x_pad, x1p, x0_self, st_in, st_ag, RG, dbg):
    # pad + cast input -> x_pad bf16  (freq +20, time +5)
    zt = big.tile([128, 1040], BF, tag="zt")
    nc.vector.memset(zt[:], 0.0)
    nc.sync.dma_start(x_pad[0:128, :], zt[:])
    nc.sync.dma_start(x_pad[128:208, :], zt[:80, :])
    for r0 in (0, 33):
        xin = work.tile([128, T], F32, tag="wrk")
        nc.sync.dma_start(xin[:], E["xs"][r0:r0 + 128, :])
        xb = work.tile([128, T], BF, tag="wrkb")
        nc.scalar.activation(xb[:], xin[:], AF.Copy)
        nc.sync.dma_start(x_pad[20 + r0:20 + r0 + 128, 5:5 + T], xb[:])

    w1 = big.tile([64, 11 * 128], BF, tag="w1e")
    for kt in range(11):
        nc.sync.dma_start(w1[:, kt * 128:(kt + 1) * 128],
                          E["w1e"][kt * 64:(kt + 1) * 64, :])
    b1 = sm.tile([128, 1], F32, tag="bias")
    nc.sync.dma_start(b1[:], E["b1e"][:, :])
    mask1 = big.tile([128, FG1 * TC], BF, tag="bigB")
    nc.sync.dma_start(mask1[:], E["mask1"][:, :])
    sel4 = sm.tile([32, 128], F32, tag="sel4")
    nc.sync.dma_start(sel4[:], E["sel4"][:, :])
    sel4T = sm.tile([128, 32], F32, tag="sel4T")
    nc.sync.dma_start(sel4T[:], E["sel4T"][:, :])

    # conv1: imf[(df), (fg, tau)] = x_pad[8fg+df, tau]; accumulate over kt
    x1 = big.tile([128, FG1 * TC], BF, tag="bigC")
    imf = big.tile([64, FG1 * 1040], BF, tag="bigA")
    nc.sync.dma_start(
        imf[:47, :].rearrange("p (fg tau) -> p fg tau", fg=FG1),
        AP(tensor=x_pad, offset=0,
           ap=[[1040, 47], [8 * 1040, FG1], [1, 1040]]))
    imv = imf[:]
    for fg in range(FG1):
        pt = psA.tile([128, 512], F32, tag="p512")
        for kt in range(11):
            rhs = AP(tensor=imf.tensor,
                     offset=imv.offset + fg * 1040 + kt,
                     ap=[[imv.ap[0][0], 47], [2, 512]])
            nc.tensor.matmul(
                out=pt[:], lhsT=w1[:47, kt * 128:(kt + 1) * 128], rhs=rhs,
                start=(kt == 0), stop=(kt == 10))
        nc.scalar.activation(x1[:, fg * TC:(fg + 1) * TC], pt[:],
                             AF.Identity, bias=b1[:])
    nc.vector.tensor_tensor(x1[:], x1[:], mask1[:], AL.mult)
    stats = sm.tile([128, 2], F32, tag="stats")
    scr = big.tile([128, FG1 * TC], BF, tag="bigA")
    nc.vector.reduce_sum(stats[:, 0:1], x1[:], AX.X)
    nc.scalar.activation(scr[:], x1[:], AF.Square, accum_out=stats[:, 1:2])
    A1, B1 = _bn_from_stats(nc, sm, psC, stats, st_in[0], st_ag[0], RG,
                            CNT1, sel4, sel4T)
    nc.vector.tensor_scalar(x1[:], x1[:], A1, B1, AL.mult, AL.add)
    nc.vector.tensor_tensor(x1[:], x1[:], mask1[:], AL.mult)
    nc.vector.tensor_scalar(x1[:], x1[:], 0.0, 20.0, AL.max, AL.min)
    if dbg:
        nc.sync.dma_start(dbg["x1f"][:, :], x1[:])

    # write x1p (zero, then 4 delta-DMAs); row = (fo+10)*32 + ci, col = t+5
    for r0 in range(0, 3584, 128):
        nc.sync.dma_start(
            AP(tensor=x1p, offset=r0 * 522, ap=[[522, 128], [1, 522]]),
            zt[:, :522])
    for d in range(4):
        sl = x1[d * 32:(d + 1) * 32, :]
        src = AP(tensor=sl.tensor, offset=sl.offset,
                 ap=[list(sl.ap[0]), [TC, FG1], [1, TC]])
        dst = AP(tensor=x1p, offset=(d + 10) * 32 * 522 + 5,
                 ap=[[522, 32], [4 * 32 * 522, FG1], [1, TC]])
        nc.sync.dma_start(dst, src)

    # conv2
    w2 = big.tile([128, 77 * 128], BF, tag="bigB")
    for k in range(77):
        nc.sync.dma_start(w2[:, k * 128:(k + 1) * 128],
                          E["w2e"][k * 128:(k + 1) * 128, :])
    b2 = sm.tile([128, 1], F32, tag="bias")
    nc.sync.dma_start(b2[:], E["b2e"][:, :])
    mask2 = big.tile([128, FG2 * TC], BF, tag="hbuf")
    nc.sync.dma_start(mask2[:], E["mask2"][:, :])

    x2 = big.tile([128, FG2 * TC], BF, tag="bigC")
    for fg in range(FG2):
        im2 = work.tile([128, 7 * 522], BF, tag="wrk")
        for c7 in range(7):
            nc.sync.dma_start(
                im2[:, c7 * 522:(c7 + 1) * 522],
                AP(tensor=x1p, offset=(8 * fg + 4 * c7) * 32 * 522,
                   ap=[[522, 128], [1, 522]]))
        pt = psA.tile([128, 512], F32, tag="p512")
        for kt in range(11):
            for c7 in range(7):
                nc.tensor.matmul(
                    out=pt[:],
                    lhsT=w2[:, (kt * 7 + c7) * 128:(kt * 7 + c7 + 1) * 128],
                    rhs=im2[:, c7 * 522 + kt:c7 * 522 + kt + 512],
                    start=(kt == 0 and c7 == 0), stop=(kt == 10 and c7 == 6))
        nc.scalar.activation(x2[:, fg * TC:(fg + 1) * TC], pt[:],
                             AF.Identity, bias=b2[:])
    nc.vector.tensor_tensor(x2[:], x2[:], mask2[:], AL.mult)
    stats2 = sm.tile([128, 2], F32, tag="stats")
    scr2 = big.tile([128, FG2 * TC], BF, tag="bigA")
    nc.vector.reduce_sum(stats2[:, 0:1], x2[:], AX.X)
    nc.scalar.activation(scr2[:], x2[:], AF.Square, accum_out=stats2[:, 1:2])
    A2, B2 = _bn_from_stats(nc, sm, psC, stats2, st_in[1], st_ag[1], RG,
                            CNT2, sel4, sel4T)
    nc.vector.tensor_scalar(x2[:], x2[:], A2, B2, AL.mult, AL.add)
    nc.vector.tensor_tensor(x2[:], x2[:], mask2[:], AL.mult)
    nc.vector.tensor_scalar(x2[:], x2[:], 0.0, 20.0, AL.max, AL.min)
    if dbg:
        nc.sync.dma_start(dbg["x2f"][:, :], x2[:])

    # x0_self [1408, 512]: row = co*41 + 4fg + d
    nc.sync.dma_start(
        AP(tensor=x0_self, offset=F0 * TC, ap=[[TC, 96], [1, TC]]),
        zt[:96, :TC])
    for d in range(4):
        nfg = 11 if d == 0 else 10
        sl = x2[d * 32:(d + 1) * 32, :]
        src = AP(tensor=sl.tensor, offset=sl.offset,
                 ap=[list(sl.ap[0]), [TC, nfg], [1, TC]])
        dst = AP(tensor=x0_self, offset=d * TC,
                 ap=[[41 * TC, 32], [4 * TC, nfg], [1, TC]])
        nc.sync.dma_start(dst, src)
        if dbg:
            nc.sync.dma_start(
                AP(tensor=dbg["x0"], offset=d * TC,
                   ap=[[41 * TC, 32], [4 * TC, nfg], [1, TC]]), src)
    if dbg:
        nc.sync.dma_start(
            AP(tensor=dbg["x0"], offset=F0 * TC, ap=[[TC, 96], [1, TC]]),
            zt[:96, :TC])


def _bn_from_stats(nc, sm, psC, stats, st_in, st_ag, RG, cnt, sel4, sel4T):
    nc.sync.dma_start(st_in[:, :], stats[:])
    nc.gpsimd.collective_compute("AllGather", AL.bypass, replica_groups=RG,
                                 ins=[st_in[:, :]], outs=[st_ag[:, :]])
    sg = sm.tile([128, 16], F32, tag="sgst")
    nc.sync.dma_start(
        sg[:].rearrange("p (co st) -> p co st", co=8),
        AP(tensor=st_ag, offset=0, ap=[[2, 128], [256, 8], [1, 2]]))
    s128 = sm.tile([128, 2], F32, tag="s128")
    sgv = sg[:]
    nc.vector.reduce_sum(
        s128[:],
        AP(tensor=sg.tensor, offset=sgv.offset,
           ap=[list(sgv.ap[0]), [1, 2], [2, 8]]), AX.X)
    pst = psC.tile([128, 128], F32, tag="psm")
    nc.tensor.matmul(out=pst[:32, 0:2], lhsT=sel4T[:], rhs=s128[:],
                     start=True, stop=True)
    stot = sm.tile([32, 2], F32, tag="stot")
    nc.scalar.activation(stot[:], pst[:32, 0:2], AF.Copy)
    mu = sm.tile([32, 4], F32, tag="mu")
    nc.vector.tensor_scalar(mu[:, 0:2], stot[:], 1.0 / cnt, None, AL.mult)
    nc.vector.tensor_tensor(mu[:, 2:3], mu[:, 0:1], mu[:, 0:1], AL.mult)
    nc.vector.tensor_tensor(mu[:, 2:3], mu[:, 1:2], mu[:, 2:3], AL.subtract)
    nc.vector.tensor_scalar(mu[:, 2:3], mu[:, 2:3], EPS, None, AL.add)
    rcp = sm.tile([32, 2], F32, tag="rcp")
    nc.vector.reciprocal(rcp[:, 0:1], mu[:, 2:3])
    nc.scalar.activation(rcp[:, 1:2], rcp[:, 0:1], AF.Sqrt)
    nc.vector.tensor_tensor(mu[:, 3:4], mu[:, 0:1], rcp[:, 1:2], AL.mult)
    nc.vector.tensor_scalar(mu[:, 3:4], mu[:, 3:4], -1.0, None, AL.mult)
    pA = psC.tile([128, 128], F32, tag="psm")
    nc.tensor.matmul(out=pA[:, 0:1], lhsT=sel4[:], rhs=rcp[:, 1:2],
                     start=True, stop=True)
    nc.tensor.matmul(out=pA[:, 1:2], lhsT=sel4[:], rhs=mu[:, 3:4],
                     start=True, stop=True)
    AB = sm.tile([128, 2], F32, tag="AB")
    nc.scalar.activation(AB[:], pA[:, 0:2], AF.Copy)
    return AB[:, 0:1], AB[:, 1:2]


# ---------------------------------------------------------------------- lstm

def _lstm(nc, tc, E, pid, big, work, ws, extp, xscp, sm, stp,
          psA, psB, psC, x0_ag, h_in, h_ag, RG, out_lp, dbg):
    slotA = (pid % 4) * HBLK
    slotB = (pid % 4) * HBLK + 4 * HBLK

    h_buf = big.tile([128, KC * TC * 2], BF, tag="hbuf")
    xw_sb = big.tile([128, NCH * TC * 2], BF, tag="bigA")
    eyeb = stp.tile([128, 128], BF, tag="eyeb")
    nc.sync.dma_start(eyeb[:], E["eyeb"][:, :])

    for l in range(min(6, PHASES)):
        is_head = (l == 5)
        nf = 11 if l == 0 else KC

        pmsb = big.tile([128, 16 * TC], BF, tag="bigB")
        for part in range(2):
            for si in range(2):
                for tck in range(4):
                    row0 = ((l * 2 + part) * 2 + si) * TC + tck * 128
                    nc.sync.dma_start(
                        pmsb[:, ((part * 2 + si) * 4 + tck) * TC:
                             ((part * 2 + si) * 4 + tck + 1) * TC],
                        E["pmats"][row0:row0 + 128, :])

        # extract + permute + BN -> xsc[(si, fc)]
        xsc = {}
        for si in range(2):
            exts = {}
            nparts = 1 if l == 0 else 2
            for part in range(nparts):
                for tb in range(4):
                    et = extp.tile([128, nf * 128], BF, tag="ext")
                    for fc in range(nf):
                        blk = work.tile([128, 128], BF, tag="eblk")
                        if l == 0:
                            off = ((pid % 4) * 2 + si) * XBLK + \
                                fc * 128 * TC + tb * 128
                            src = AP(tensor=x0_ag, offset=off,
                                     ap=[[TC, 128], [1, 128]])
                        else:
                            base = slotA if part == 0 else slotB
                            off = base + fc * 1024 + si * TC + tb * 128
                            src = AP(tensor=h_ag[l - 1], offset=off,
                                     ap=[[KC * TC * 2, 128], [1, 128]])
                        nc.sync.dma_start(blk[:], src)
                        ptp = psC.tile([128, 128], BF, tag="psm")
                        nc.tensor.transpose(out=ptp[:], in_=blk[:],
                                            identity=eyeb[:])
                        nc.scalar.activation(
                            et[:, fc * 128:(fc + 1) * 128], ptp[:], AF.Copy)
                    exts[(part, tb)] = et
            for fc in range(nf):
                pp = psA.tile([128, 512], F32, tag="p512")
                for part in range(nparts):
                    for tb in range(4):
                        nc.tensor.matmul(
                            out=pp[:],
                            lhsT=exts[(part, tb)][:, fc * 128:(fc + 1) * 128],
                            rhs=pmsb[:, ((part * 2 + si) * 4 + tb) * TC:
                                     ((part * 2 + si) * 4 + tb + 1) * TC],
                            start=(part == 0 and tb == 0),
                            stop=(part == nparts - 1 and tb == 3))
                xt = xscp.tile([128, 512], BF, tag="xsc")
                if l == 0:
                    nc.scalar.activation(xt[:], pp[:], AF.Copy)
                else:
                    Ab = sm.tile([128, 2], F32, tag="bnab")
                    nc.sync.dma_start(
                        Ab[:, 0:1],
                        AP(tensor=E["bnA"], offset=(l - 1) * HID + fc * 128,
                           ap=[[1, 128], [1, 1]]))
                    nc.sync.dma_start(
                        Ab[:, 1:2],
                        AP(tensor=E["bnB"], offset=(l - 1) * HID + fc * 128,
                           ap=[[1, 128], [1, 1]]))
                    nc.scalar.activation(xt[:], pp[:], AF.Identity,
                                         bias=Ab[:, 1:2], scale=Ab[:, 0:1])
                xsc[(si, fc)] = xt

            if is_head:
                _head_one(nc, E, work, sm, psA, psC, xsc, si, out_lp)
                for fc in range(nf):
                    del xsc[(si, fc)]
                continue

            # projection for this sample -> xw_sb
            for n in range(NCH):
                wt = ws.tile([128, nf * 128], BF, tag="wi")
                if l == 0:
                    src = AP(tensor=E["wi0"], offset=n * 128,
                             ap=[[G, 128], [128 * G, nf], [1, 128]])
                else:
                    src = AP(tensor=E["wi"],
                             offset=((l - 1) * HID) * G + n * 128,
                             ap=[[G, 128], [128 * G, nf], [1, 128]])
                nc.sync.dma_start(wt[:], src)
                bln = sm.tile([128, 1], F32, tag="bias")
                nc.sync.dma_start(
                    bln[:], AP(tensor=E["bl"], offset=l * G + n * 128,
                               ap=[[1, 128], [1, 1]]))
                pj = psA.tile([128, 512], F32, tag="p512")
                for fc in range(nf):
                    nc.tensor.matmul(
                        out=pj[:], lhsT=wt[:, fc * 128:(fc + 1) * 128],
                        rhs=xsc[(si, fc)][:], start=(fc == 0),
                        stop=(fc == nf - 1))
                nc.scalar.activation(
                    xw_sb[:, n * 1024 + si * TC:n * 1024 + (si + 1) * TC],
                    pj[:], AF.Identity, bias=bln[:])
            for fc in range(nf):
                del xsc[(si, fc)]

        if is_head:
            return
        if dbg:
            nc.sync.dma_start(dbg["xw"][l * 128:(l + 1) * 128, :], xw_sb[:])

        # wh resident
        whsb = big.tile([128, KC * G], BF, tag="bigB")
        for k in range(KC):
            nc.sync.dma_start(
                whsb[:, k * G:(k + 1) * G],
                AP(tensor=E["wh"], offset=(l * HID + k * 128) * G,
                   ap=[[G, 128], [1, G]]))

        # recurrent scan
        ha = stp.tile([128, 12], BF, tag="ha")
        hb = stp.tile([128, 12], BF, tag="hb")
        ca = stp.tile([128, 12], F32, tag="ca")
        cb = stp.tile([128, 12], F32, tag="cb")
        nc.vector.memset(ha[:], 0.0)
        nc.vector.memset(ca[:], 0.0)
        xwv = xw_sb[:]
        hv = h_buf[:]
        with tc.For_i(0, TC // U, hint_engines=(ET.PE,)) as i:
            for u in range(U):
                hc, hn = (ha, hb) if u % 2 == 0 else (hb, ha)
                cc, cn = (ca, cb) if u % 2 == 0 else (cb, ca)
                ps = psB.tile([128, 48], F32, tag="gates")
                for ccg in range(KC):
                    for g in range(4):
                        j = g * KC + ccg
                        for k in range(KC):
                            nc.tensor.matmul(
                                out=ps[:, 2 * j:2 * j + 2],
                                lhsT=whsb[:, k * G + j * 128:
                                          k * G + (j + 1) * 128],
                                rhs=hc[:, 2 * k:2 * k + 2],
                                start=(k == 0), stop=(k == KC - 1))
                    gt = sm.tile([128, 8], F32, tag="gt")
                    psAP = AP(tensor=ps.tensor, offset=ps[:].offset + 2 * ccg,
                              ap=[list(ps[:].ap[0]), [2 * KC, 4], [1, 2]])
                    xwAP = AP(tensor=xw_sb.tensor,
                              offset=xwv.offset + ccg * 1024 + i * U + u,
                              ap=[list(xwv.ap[0]), [KC * 1024, 4], [TC, 2]])
                    nc.vector.tensor_tensor(
                        gt[:].rearrange("p (a c) -> p a c", a=4), psAP, xwAP,
                        AL.add)
                    sg = sm.tile([128, 6], F32, tag="sg")
                    nc.scalar.activation(sg[:], gt[:, 0:6], AF.Sigmoid)
                    tg = sm.tile([128, 2], F32, tag="tg")
                    nc.scalar.activation(tg[:], gt[:, 6:8], AF.Tanh)
                    t1 = sm.tile([128, 2], F32, tag="t1")
                    nc.vector.tensor_tensor(t1[:], sg[:, 2:4],
                                            cc[:, 2 * ccg:2 * ccg + 2], AL.mult)
                    t2 = sm.tile([128, 2], F32, tag="t2")
                    nc.vector.tensor_tensor(t2[:], sg[:, 0:2], tg[:], AL.mult)
                    nc.vector.tensor_tensor(cn[:, 2 * ccg:2 * ccg + 2],
                                            t1[:], t2[:], AL.add)
                    tc2 = sm.tile([128, 2], F32, tag="tc2")
                    nc.scalar.activation(tc2[:], cn[:, 2 * ccg:2 * ccg + 2],
                                         AF.Tanh)
                    nc.vector.tensor_tensor(hn[:, 2 * ccg:2 * ccg + 2],
                                            sg[:, 4:6], tc2[:], AL.mult)
                dst = AP(tensor=h_buf.tensor,
                         offset=hv.offset + i * U + u,
                         ap=[list(hv.ap[0]), [1024, KC], [TC, 2]])
                nc.gpsimd.tensor_copy(
                    dst, hn[:].rearrange("p (k s) -> p k s", k=KC))

        nc.sync.dma_start(h_in[:, :], h_buf[:])
        if dbg:
            nc.sync.dma_start(dbg["hb"][l * 128:(l + 1) * 128, :], h_buf[:])
        nc.gpsimd.collective_compute(
            "AllGather", AL.bypass, replica_groups=RG,
            ins=[h_in[:, :]], outs=[h_ag[l][:, :]])


def _head_one(nc, E, work, sm, psA, psC, xsc, si, out_lp):
    fcsb = sm.tile([128, KC * 32], BF, tag="fcsb")
    for fc in range(KC):
        nc.sync.dma_start(fcsb[:, fc * 32:(fc + 1) * 32],
                          E["fcw"][fc * 128:(fc + 1) * 128, :])
    eye = sm.tile([32, 32], F32, tag="eye32")
    nc.sync.dma_start(eye[:], E["eye32"][:, :])

    pf = psA.tile([32, 512], F32, tag="p512")
    for fc in range(KC):
        nc.tensor.matmul(out=pf[:], lhsT=fcsb[:, fc * 32:(fc + 1) * 32],
                         rhs=xsc[(si, fc)][:], start=(fc == 0),
                         stop=(fc == KC - 1))
    lg = work.tile([32, 512], F32, tag="lg")
    nc.scalar.activation(lg[:], pf[:], AF.Copy)
    for tb4 in range(4):
        tp = psC.tile([128, 128], F32, tag="psm")
        nc.tensor.transpose(out=tp[:, 0:32], in_=lg[:, tb4 * 128:(tb4 + 1) * 128],
                            identity=eye[:])
        lgt = sm.tile([128, 32], F32, tag="lgt")
        nc.scalar.activation(lgt[:], tp[:, 0:32], AF.Copy)
        mx = sm.tile([128, 4], F32, tag="mx")
        nc.vector.reduce_max(mx[:, 0:1], lgt[:, 0:NCLS], AX.X)
        nc.vector.tensor_scalar(mx[:, 1:2], mx[:, 0:1], -1.0, None, AL.mult)
        et = sm.tile([128, 32], F32, tag="et")
        nc.scalar.activation(et[:, 0:NCLS], lgt[:, 0:NCLS], AF.Exp,
                             bias=mx[:, 1:2], accum_out=mx[:, 2:3])
        nc.scalar.activation(mx[:, 3:4], mx[:, 2:3], AF.Ln)
        tot = sm.tile([128, 1], F32, tag="tot")
        nc.vector.tensor_tensor(tot[:], mx[:, 0:1], mx[:, 3:4], AL.add)
        ot = sm.tile([128, 32], F32, tag="ot")
        nc.vector.tensor_scalar(ot[:, 0:NCLS], lgt[:, 0:NCLS], tot[:], None,
                                AL.subtract)
        nc.sync.dma_start(
            AP(tensor=out_lp, offset=(tb4 * 128 * 2 + si) * NCLS,
               ap=[[2 * NCLS, 128], [1, NCLS]]),
            ot[:, 0:NCLS])


# -------------------------------------------------------------------- kernel

def _get_program(debug=False):
    key = ("dbg" if debug else "main")
    if key not in _CACHE:
        _CACHE[key] = _build_program(debug=debug)
    return _CACHE[key]


def kernel(**inputs):
    in_maps, out_len = _host_pack(inputs)
    nc = _get_program(debug=False)
    res = run_bass_kernel_spmd(nc, in_maps, core_ids=list(range(NC_)))
    lp = np.zeros((B, TC, NCLS), np.float32)
    for c in range(4):
        o = res.results[c]["out_lp"].reshape(TC, 2, NCLS)
        lp[2 * c] = o[:, 0]
        lp[2 * c + 1] = o[:, 1]
    return lp, out_len
